# revision 16
# baseline (speedup 1.0000x reference)
"""Trainium2 Bass kernel for nn_HNL_90185723281715 (scatter_memory).

Computation (see reference):
  q = x @ W_q.T                     [B, H, D]
  q_hat = q / ||q||                 (L2 over D)
  m_hat = memories / ||memories||   (L2 over D)
  s = q_hat . m_hat                 [B, H, M]   (cosine scores, in [-1, 1])
  p = softmax(s)                    (T=1; max-subtraction skipped -- s bounded)
  out = (p @ m_hat) * sqrt(D)       [B, H*D]

Sharding: tensor-parallel over heads -- 2 heads per core, full batch on
every core. x is replicated (host pre-transposed); W_q and memories are
sliced per head-pair. Output is gathered/transposed on host.

All matmul operands are float32r (fp32 storage, single-pass PE matmul at
bf16 rate); producers write fp32r so walrus' rounding check passes.
"""

import numpy as np
from contextlib import ExitStack

import concourse.bacc as bacc
import concourse.tile as tile
from concourse import mybir
from concourse.bass_utils import run_bass_kernel_spmd
from concourse.masks import make_identity

F32 = mybir.dt.float32
F32R = mybir.dt.float32r
MMDT = F32R   # dtype for matmul operand tiles

B = 4096          # batch rows
IN = 1024         # in features
H = 16            # heads total
M = 4096          # memories per head
D = 64            # head dim
N_CORES = 8
HPC = H // N_CORES    # 2 heads per core
G = 512               # rows per group
NG = B // G           # 8 row groups
NCH = M // 128        # 32 mem chunks of 128


def emit(tc, ctx, xT, wqT, mem, outT, blkA_in, blkB_in, repeat, dbg=None):
    nc = tc.nc
    ctx.enter_context(
        nc.allow_low_precision(reason="float32r matmul operands (fp32 container)")
    )

    const = ctx.enter_context(tc.tile_pool(name="const", bufs=1))
    persist = ctx.enter_context(tc.tile_pool(name="persist", bufs=1))
    xpool = ctx.enter_context(tc.tile_pool(name="xk", bufs=2))
    expool = ctx.enter_context(tc.tile_pool(name="exp", bufs=3))
    small = ctx.enter_context(tc.tile_pool(name="small", bufs=1))
    small2 = ctx.enter_context(tc.tile_pool(name="small2", bufs=2))

    # --- constants ---
    ident = const.tile([128, 128], F32)
    make_identity(nc, ident[:])
    onesf = const.tile([128, D], F32)
    nc.vector.memset(onesf[:], 1.0)
    blkT = const.tile([2, 128], MMDT)
    nc.sync.dma_start(blkT[:], blkB_in)
    blkones = const.tile([128, 2], MMDT)
    nc.sync.dma_start(blkones[:], blkA_in)

    # W_q slice for this core's two heads, pre-transposed: wqT [IN, 128]
    wq = const.tile([128, 8, 128], MMDT)   # [k-partition, k-chunk, feat]
    nc.sync.dma_start(wq[:], wqT.rearrange("(k p) f -> p k f", p=128))

    for _ in range(repeat):
        # =========== memories: load, normalize, build ones column ==========
        # memn1[h]: [128 mems, 32 chunks, 64 dims + ones col]  (mm2 weights)
        memn1 = []
        for h in range(HPC):
            mt = persist.tile([128, NCH, D + 1], MMDT, tag=f"memn1_{h}")
            nc.sync.dma_start(
                mt[:, :, 0:D], mem[h].rearrange("(c p) d -> p c d", p=128)
            )
            nc.vector.tensor_copy(mt[:, :, D], onesf[:, 0:NCH])
            memn1.append(mt)

        for h in range(HPC):
            sq = small.tile([128, NCH, D], F32, tag="msq")
            nc.scalar.square(sq[:], memn1[h][:, :, 0:D].bitcast(F32))
            ssq = small.tile([128, NCH], F32, tag="mssq")
            nc.vector.reduce_sum(ssq[:], sq[:], axis=mybir.AxisListType.X)
            mnorm = small.tile([128, NCH], F32, tag="mnorm")
            nc.scalar.sqrt(mnorm[:], ssq[:])
            minv = small.tile([128, NCH], F32, tag="minv")
            nc.vector.reciprocal(minv[:], mnorm[:])
            for c in range(NCH):
                nc.vector.tensor_scalar_mul(
                    memn1[h][:, c, 0:D], memn1[h][:, c, 0:D], minv[:, c : c + 1]
                )

        # =========== memT: [128 (2 heads x 64 dims), 4096 mems] ============
        # PE-transpose normalized [128 mems, 64] chunks -> [64, 128 mems].
        memT0 = persist.tile([D, M], MMDT, tag="memT0")
        memT1 = persist.tile([D, M], MMDT, tag="memT1")
        memT = [memT0, memT1]
        with tc.tile_pool(name="ptr", bufs=2, space="PSUM") as ptrp:
            for c4 in range(NCH // 4):
                for h in range(HPC):
                    pt = ptrp.tile([D, 512], F32, tag=f"ptr{h}")
                    for j in range(4):
                        c = c4 * 4 + j
                        nc.tensor.transpose(
                            pt[:, j * 128 : (j + 1) * 128],
                            memn1[h][:, c, 0:D].bitcast(F32),
                            ident[:],
                        )
                    nc.vector.tensor_copy(
                        memT[h][:, c4 * 512 : (c4 + 1) * 512], pt[:]
                    )

        # =========== q projection: qT = wq_slice @ x.T  [128, B] ===========
        qsq = persist.tile([128, B], MMDT, tag="qsq")
        qT = persist.tile([128, B], MMDT, tag="qT")
        qinv = persist.tile([2, B], MMDT, tag="qinv")
        with tc.tile_pool(name="pq", bufs=1, space="PSUM") as pqp:
            pq = pqp.tile([128, B], F32, tag="pq")
            for k in range(8):
                xk = xpool.tile([128, B], MMDT, tag="xk")
                nc.sync.dma_start(xk[:], xT[k * 128 : (k + 1) * 128, :])
                for j in range(NG):
                    nc.tensor.matmul(
                        pq[:, j * G : (j + 1) * G],
                        wq[:, k, :],
                        xk[:, j * G : (j + 1) * G],
                        start=(k == 0),
                        stop=(k == 7),
                    )

            # q norms: sumsq via ones-matmul over each head's 64 partitions.
            nc.scalar.square(qsq[:], pq[:])
            nc.vector.tensor_copy(qT[:], pq[:])

            pns = pqp.tile([2, B], F32, tag="pq")
            for j in range(NG):
                nc.tensor.matmul(
                    pns[:, j * G : (j + 1) * G],
                    blkones[:],
                    qsq[:, j * G : (j + 1) * G],
                    start=True,
                    stop=True,
                )
            nc.scalar.sqrt(qinv[:], pns[:])
            nc.vector.reciprocal(qinv[:], qinv[:])

            # qbc[p, r] = qinv[head(p), r] via K=2 selector matmul
            qbcp = pqp.tile([128, B], F32, tag="pq")
            for j in range(NG):
                nc.tensor.matmul(
                    qbcp[:, j * G : (j + 1) * G],
                    blkT[:],
                    qinv[:, j * G : (j + 1) * G],
                    start=True,
                    stop=True,
                )
            nc.vector.tensor_mul(qT[:], qT[:], qbcp[:].bitcast(F32R))
        # head B rows shifted to partitions 0-63 (fp32r matmuls need base 0)
        qh1 = persist.tile([D, B], MMDT, tag="qsq2")
        nc.sync.dma_start(qh1[:], qT[64:128, :])
        qrhs = [qT[0:64, :], qh1[:]]

        if dbg is not None:
            nc.sync.dma_start(dbg["qt"], qT[:].bitcast(F32))
            nc.sync.dma_start(dbg["qbc"], qbc[:].bitcast(F32))
            nc.sync.dma_start(dbg["qh1"], qh1[:].bitcast(F32))
            nc.sync.dma_start(dbg["memT1"], memT[1][:].bitcast(F32))
            nc.sync.dma_start(dbg["qinv"], qinv[:].bitcast(F32))

        # =========== main loop: scores -> exp -> combine ===================
        with (
            tc.tile_pool(name="sc", bufs=3, space="PSUM") as scp,
            tc.tile_pool(name="acc", bufs=2, space="PSUM") as accp,
        ):
            for g in range(NG):
                gs = slice(g * G, (g + 1) * G)
                for h in range(HPC):
                    acc = accp.tile([D + 1, G], F32, tag="acc")
                    for cp in range(NCH // 2):
                        sc = scp.tile([128, 1024], F32, tag="sc")
                        for i in range(2):
                            c = cp * 2 + i
                            nc.tensor.matmul(
                                sc[:, i * G : (i + 1) * G],
                                memT[h][:, c * 128 : (c + 1) * 128],
                                qrhs[h][:, gs],
                                start=True,
                                stop=True,
                            )
                        ex = expool.tile([128, 1024], MMDT, tag="exp")
                        nc.scalar.activation(
                            ex[:], sc[:], mybir.ActivationFunctionType.Exp
                        )
                        for i in range(2):
                            c = cp * 2 + i
                            nc.tensor.matmul(
                                acc[:],
                                memn1[h][:, c, :],
                                ex[:, i * G : (i + 1) * G],
                                start=(c == 0),
                                stop=(c == NCH - 1),
                            )
                    # finalize: out = acc[0:D] * (sqrt(D) / denom)
                    dinv = small2.tile([1, G], F32, tag="dinv")
                    nc.vector.reciprocal(dinv[:], acc[D : D + 1, :])
                    nc.vector.tensor_scalar_mul(dinv[:], dinv[:], float(np.sqrt(D)))
                    bc = small2.tile([D, G], F32, tag="bc")
                    nc.gpsimd.partition_broadcast(bc[:], dinv[:])
                    ostage = small2.tile([D, G], F32, tag="ostage")
                    nc.vector.tensor_mul(ostage[:], acc[0:D, :], bc[:])
                    nc.sync.dma_start(outT[h * D : (h + 1) * D, gs], ostage[:])


def build(repeat=1, debug_dump=False):
    nc = bacc.Bacc(
        "TRN2", target_bir_lowering=False, debug=False, num_devices=N_CORES
    )
    xT_ap = nc.dram_tensor("xT", [IN, B], MMDT, kind="ExternalInput").ap()
    wqT_ap = nc.dram_tensor("wqT", [IN, 128], MMDT, kind="ExternalInput").ap()
    mem_ap = nc.dram_tensor("mem", [HPC, M, D], MMDT, kind="ExternalInput").ap()
    outT_ap = nc.dram_tensor("outT", [128, B], F32, kind="ExternalOutput").ap()
    blkA_ap = nc.dram_tensor("blkA", [128, 2], MMDT, kind="ExternalInput").ap()
    blkB_ap = nc.dram_tensor("blkB", [2, 128], MMDT, kind="ExternalInput").ap()
    dbg = None
    if debug_dump:
        dbg = {
            "qt": nc.dram_tensor("dbg_qt", [128, B], F32, kind="ExternalOutput").ap(),
            "qbc": nc.dram_tensor("dbg_qbc", [128, B], F32, kind="ExternalOutput").ap(),
            "qh1": nc.dram_tensor("dbg_qh1", [D, B], F32, kind="ExternalOutput").ap(),
            "memT1": nc.dram_tensor("dbg_memT1", [D, M], F32, kind="ExternalOutput").ap(),
            "qinv": nc.dram_tensor("dbg_qinv", [2, B], F32, kind="ExternalOutput").ap(),
        }
    with tile.TileContext(nc) as tc, ExitStack() as ctx:
        emit(tc, ctx, xT_ap, wqT_ap, mem_ap, outT_ap, blkA_ap, blkB_ap, repeat, dbg)
    nc.compile()
    return nc


BLK_A = np.zeros((128, 2), np.float32)
BLK_A[0:64, 0] = 1.0
BLK_A[64:128, 1] = 1.0
BLK_B = np.ascontiguousarray(BLK_A.T)


def run(x, W_q, memories, repeat=1, nc=None):
    if nc is None:
        nc = build(repeat)
    xT = np.ascontiguousarray(x.T)
    in_maps = []
    for i in range(N_CORES):
        in_maps.append(
            {
                "xT": xT,
                "wqT": np.ascontiguousarray(W_q[i * 128 : (i + 1) * 128, :].T),
                "mem": np.ascontiguousarray(memories[i * HPC : (i + 1) * HPC]),
                "blkA": BLK_A,
                "blkB": BLK_B,
            }
        )
    res = run_bass_kernel_spmd(nc, in_maps, list(range(N_CORES)))
    out = np.empty((B, H * D), dtype=np.float32)
    for i in range(N_CORES):
        out[:, i * 128 : (i + 1) * 128] = res.results[i]["outT"].T
    return out


def kernel(x, W_q, memories):
    return run(x, W_q, memories)
